# revision 12
# baseline (speedup 1.0000x reference)
"""RNN-T Joiner kernel for Trainium2 (Bass/Tile), 8-core data-parallel over batch.

out[b,t,u,v] = (enc[b,t] @ We)[v] + (pred[b,u] @ Wp)[v] + bias[v]

Layout trick: V on partitions, (u, t) on the free dim. The broadcast add is
done by ONE DVE tensor_tensor per u-half with stride-0 broadcast access
patterns ([128, nu, 32, 8]: enc broadcast over u, pred_rep broadcast over
t-blocks, both with packed 2-byte last dims so the DVE 2x mode stays on).
This amortizes the ~120 ns per-instruction overhead over ~8k elements.
GpSimd is deliberately unused: its tensor_scalar is ~4 us/instr on HW and
its SBUF-port contention degrades DVE ~7x (measured).

All inputs are host-packed so the whole load phase is 12 large contiguous
DMAs (the per-vt fused We|Wp block first, so PE can start ~8 us earlier than
with per-chunk loads; the SP sequencer costs ~0.6 us per dma_start).

Per core (one batch element):
  - PE (bf16): per vt, pred_projT [v,65] (+bias rank-1) and enc_projT [v,256]
    accumulated in PSUM f32.
  - Act: PSUM->SBUF bf16 copies: enc_projT [128,256] and pred_rep [128,65,8]
    (each pred value replicated 8x so the TT inner dim stays packed).
  - DVE: one tensor_tensor add per u-half (16 total, ~4.4 us each at 2x).
  - HWDGE DMA: 8 stores of ~4.3 MB ([v,u,t]-order bf16 output).

Output returned to DRAM as bf16 [V, U1*T]; host transposes to [T,U1,V] f32.
bf16 end-to-end keeps max rel err ~5e-3, well under the 2e-2 gate.
"""

import sys

sys.path.insert(0, "/opt/trn_rl_repo")

import numpy as np

B, T, U1, D, V = 8, 256, 65, 640, 1024
KC = D // 128   # 5 contraction chunks
NVT = V // 128  # 8 vocab tiles
HALVES = [(0, 33), (33, 32)]

_COMPILED = None


def _build():
    import concourse.bacc as bacc
    import concourse.tile as tile
    import concourse.mybir as mybir

    f32 = mybir.dt.float32
    bf16 = mybir.dt.bfloat16

    nc = bacc.Bacc("TRN2", target_bir_lowering=False, debug=False, num_devices=8)

    # host-packed inputs (see _in_maps):
    #   encTp[p, c*256+t] = enc[t, c*128+p]
    #   predTp[p, c*65+u] = pred[u, c*128+p]
    #   Wall[vt*128+p, c*128+j]       = W[c*128+p? ...] enc half (see _pack_w)
    #   Wall[vt*128+p, 640+c*128+j]   = pred half
    encTp = nc.dram_tensor("encTp", [128, KC * T], bf16, kind="ExternalInput")
    predTp = nc.dram_tensor("predTp", [128, KC * U1], bf16, kind="ExternalInput")
    Wall = nc.dram_tensor("Wall", [NVT * 128, 2 * KC * 128], bf16,
                          kind="ExternalInput")
    bias = nc.dram_tensor("bias", [1, V], bf16, kind="ExternalInput")
    ones = nc.dram_tensor("ones", [1, U1], bf16, kind="ExternalInput")
    out = nc.dram_tensor("out", [V, U1 * T], bf16, kind="ExternalOutput")

    with tile.TileContext(nc) as tc:
        with tc.tile_pool(name="consts", bufs=1) as cp:
            # split load issue across the two HWDGE sequencers: weights on SP,
            # activations/bias on Act (the SP sequencer costs ~0.7 us per
            # dma_start, so a single 12-deep chain would delay the pipeline)
            W_sb = []
            for vt in range(NVT):
                t_ = cp.tile([128, 2 * KC * 128], bf16, tag=f"W{vt}")
                W_sb.append(t_)
            nc.sync.dma_start(W_sb[0][:], Wall[0:128, :])
            predT_sb = cp.tile([128, KC * U1], bf16, tag="predT")
            nc.scalar.dma_start(predT_sb[:], predTp[:])
            encT_sb = cp.tile([128, KC * T], bf16, tag="encT")
            nc.scalar.dma_start(encT_sb[:], encTp[:])
            bias_sb = cp.tile([1, V], bf16, tag="bias")
            nc.scalar.dma_start(bias_sb[:], bias[:])
            ones_sb = cp.tile([1, U1], bf16, tag="ones")
            nc.scalar.dma_start(ones_sb[:], ones[:])
            for vt in range(1, NVT):
                nc.sync.dma_start(W_sb[vt][:], Wall[vt * 128:(vt + 1) * 128, :])

            enc_sb = cp.tile([128, NVT * T], bf16, tag="enc_sb")

            with tc.tile_pool(name="ppsum", bufs=2, space="PSUM") as pp, \
                 tc.tile_pool(name="epsum", bufs=2, space="PSUM") as ep, \
                 tc.tile_pool(name="rep", bufs=3) as rp, \
                 tc.tile_pool(name="stage", bufs=3) as sp:
                for vt in range(NVT):
                    vs = slice(vt * 128, (vt + 1) * 128)
                    # pred_projT[v, u] = pred[u] @ Wp[:, v] + bias[v]
                    psp = pp.tile([128, U1], f32, tag="pp")
                    for c in range(KC):
                        nc.tensor.matmul(
                            psp[:], W_sb[vt][:, 640 + c * 128:640 + (c + 1) * 128],
                            predT_sb[:, c * U1:(c + 1) * U1],
                            start=(c == 0), stop=False)
                    nc.tensor.matmul(
                        psp[:], bias_sb[0:1, vs], ones_sb[0:1, :],
                        start=False, stop=True)
                    # pred_rep[v, u, r] = pred_projT[v, u] replicated 8x
                    rep = rp.tile([128, U1 * 8], bf16, tag="rep")
                    nc.scalar.copy(
                        rep[:].rearrange("p (u r) -> p u r", r=8),
                        psp[:].unsqueeze(2).broadcast_to([128, U1, 8]))

                    # enc_projT[v, t] = enc[t] @ We[:, v]
                    pse = ep.tile([128, T], f32, tag="pse")
                    for c in range(KC):
                        nc.tensor.matmul(
                            pse[:], W_sb[vt][:, c * 128:(c + 1) * 128],
                            encT_sb[:, c * T:(c + 1) * T],
                            start=(c == 0), stop=(c == KC - 1))
                    esl = enc_sb[:, vt * T:(vt + 1) * T]
                    nc.scalar.copy(esl, pse[:])

                    # broadcast-add + store: one TT + one ~2.1 MB store per half
                    for (u0, nu) in HALVES:
                        st = sp.tile([128, 33 * T], bf16, tag="stage")
                        in0 = (esl.rearrange("p (b r) -> p b r", r=8)
                               .unsqueeze(1).broadcast_to([128, nu, 32, 8]))
                        in1 = (rep[:, u0 * 8:(u0 + nu) * 8]
                               .rearrange("p (u r) -> p u r", r=8)
                               .unsqueeze(2).broadcast_to([128, nu, 32, 8]))
                        outp = st[:, 0:nu * T].rearrange(
                            "p (u b r) -> p u b r", u=nu, b=32, r=8)
                        nc.vector.tensor_add(outp, in0, in1)
                        nc.sync.dma_start(
                            out[vs, u0 * T:(u0 + nu) * T], st[:, 0:nu * T])

    nc.compile()
    return nc


def _get_compiled():
    global _COMPILED
    if _COMPILED is None:
        _COMPILED = _build()
    return _COMPILED


def _bf16(a):
    import ml_dtypes
    return np.ascontiguousarray(a.astype(ml_dtypes.bfloat16))


def _pack_chunks(xT, cols):
    # xT: [D, cols] -> [128, KC*cols] with [p, c*cols+i] = xT[c*128+p, i]
    return xT.reshape(KC, 128, cols).transpose(1, 0, 2).reshape(128, KC * cols)


def _pack_w(Whalf):
    # [c*128+d', vt*128+j] -> [vt*128+d', c*128+j]
    w = np.asarray(Whalf).reshape(KC, 128, NVT, 128)
    return w.transpose(2, 1, 0, 3).reshape(NVT * 128, KC * 128)


def _in_maps(encoder_out, predictor_out, W, b):
    W = np.asarray(W)
    Wall = _bf16(np.concatenate([_pack_w(W[:D]), _pack_w(W[D:])], axis=1))
    bias = _bf16(np.asarray(b).reshape(1, V))
    ones = _bf16(np.ones((1, U1), dtype=np.float32))
    maps = []
    for i in range(B):
        maps.append({
            "encTp": _bf16(_pack_chunks(np.asarray(encoder_out[i]).T, T)),
            "predTp": _bf16(_pack_chunks(np.asarray(predictor_out[i]).T, U1)),
            "Wall": Wall,
            "bias": bias,
            "ones": ones,
        })
    return maps


def run(encoder_out, predictor_out, W, b, trace=False, tmpdir=None):
    from concourse.bass_utils import run_bass_kernel_spmd

    nc = _get_compiled()
    maps = _in_maps(encoder_out, predictor_out, W, b)
    res = run_bass_kernel_spmd(
        nc, maps, list(range(B)), trace=trace,
        **({"tmpdir": tmpdir} if tmpdir else {}))
    outs = np.empty((B, T, U1, V), dtype=np.float32)
    for i in range(B):
        o = np.asarray(res.results[i]["out"])
        o16 = o.view(np.uint16).reshape(V, U1, T)
        f = (o16.astype(np.uint32) << np.uint32(16)).view(np.float32)
        outs[i] = f.transpose(2, 1, 0)
    return outs, res


def kernel(encoder_out, predictor_out, W, b):
    outs, _ = run(encoder_out, predictor_out, W, b)
    return outs


# revision 16
# speedup vs baseline: 1.0001x; 1.0001x over previous
"""RNN-T Joiner kernel for Trainium2 (Bass/Tile), 8-core data-parallel over batch.

out[b,t,u,v] = (enc[b,t] @ We)[v] + (pred[b,u] @ Wp)[v] + bias[v]

Layout trick: V on partitions, (u, t) on the free dim. The broadcast add is
done by ONE DVE tensor_tensor per u-half with stride-0 broadcast access
patterns ([128, nu, 32, 8]: enc broadcast over u, pred_rep broadcast over
t-blocks, both with packed 2-byte last dims so the DVE 2x mode stays on).
This amortizes the ~120 ns per-instruction overhead over ~8k elements.
GpSimd is deliberately unused: its tensor_scalar is ~4 us/instr on HW and
its SBUF-port contention degrades DVE ~7x (measured).

All inputs are host-packed so the whole load phase is 12 large contiguous
DMAs (the per-vt fused We|Wp block first, so PE can start ~8 us earlier than
with per-chunk loads; the SP sequencer costs ~0.6 us per dma_start).

Per core (one batch element):
  - PE (bf16): per vt, pred_projT [v,65] (+bias rank-1) and enc_projT [v,256]
    accumulated in PSUM f32.
  - Act: PSUM->SBUF bf16 copies: enc_projT [128,256] and pred_rep [128,65,8]
    (each pred value replicated 8x so the TT inner dim stays packed).
  - DVE: one tensor_tensor add per u-half (16 total, ~4.4 us each at 2x).
  - HWDGE DMA: 8 stores of ~4.3 MB ([v,u,t]-order bf16 output).

Output returned to DRAM as bf16 [V, U1*T]; host transposes to [T,U1,V] f32.
bf16 end-to-end keeps max rel err ~5e-3, well under the 2e-2 gate.
"""

import sys

sys.path.insert(0, "/opt/trn_rl_repo")

import numpy as np

B, T, U1, D, V = 8, 256, 65, 640, 1024
KC = D // 128   # 5 contraction chunks
NVT = V // 128  # 8 vocab tiles
HALVES = [(0, 33), (33, 32)]

_COMPILED = None


def _build():
    import concourse.bacc as bacc
    import concourse.tile as tile
    import concourse.mybir as mybir

    f32 = mybir.dt.float32
    bf16 = mybir.dt.bfloat16

    nc = bacc.Bacc("TRN2", target_bir_lowering=False, debug=False, num_devices=8)

    # host-packed inputs (see _in_maps):
    #   encTp[p, c*256+t] = enc[t, c*128+p]
    #   predTp[p, c*65+u] = pred[u, c*128+p]
    #   Wall[vt*128+p, c*128+j]       = W[c*128+p? ...] enc half (see _pack_w)
    #   Wall[vt*128+p, 640+c*128+j]   = pred half
    # smalls: predT packed [128, KC*U1], then (row 0 only) bias [1, V] at cols
    # PB0..PB0+V and ones [1, U1] at cols OB0..OB0+U1
    encTp = nc.dram_tensor("encTp", [128, KC * T], bf16, kind="ExternalInput")
    smalls = nc.dram_tensor("smalls", [128, KC * U1 + V + U1], bf16,
                            kind="ExternalInput")
    Wall = nc.dram_tensor("Wall", [NVT * 128, 2 * KC * 128], bf16,
                          kind="ExternalInput")
    out = nc.dram_tensor("out", [V, U1 * T], bf16, kind="ExternalOutput")
    PB0 = KC * U1
    OB0 = PB0 + V

    with tile.TileContext(nc) as tc:
        with tc.tile_pool(name="consts", bufs=1) as cp:
            # ALL DMAs stay on the SP HWDGE ring: issuing loads from the Act
            # ring measurably unbalances the SDMA engines (engine 15 ran ~20%
            # slower for the whole kernel, costing ~10 us end-to-end).
            # Load order: smalls (pred path + bias), W0, encT, W1-7 — the SP
            # sequencer costs ~0.6 us per dma_start, so the first-TT path
            # depends only on the first three issues.
            W_sb = []
            for vt in range(NVT):
                t_ = cp.tile([128, 2 * KC * 128], bf16, tag=f"W{vt}")
                W_sb.append(t_)
            sm_sb = cp.tile([128, KC * U1 + V + U1], bf16, tag="smalls")
            nc.sync.dma_start(sm_sb[:], smalls[:])
            nc.sync.dma_start(W_sb[0][:], Wall[0:128, :])
            encT_sb = cp.tile([128, KC * T], bf16, tag="encT")
            nc.sync.dma_start(encT_sb[:], encTp[:])
            for vt in range(1, NVT):
                nc.sync.dma_start(W_sb[vt][:], Wall[vt * 128:(vt + 1) * 128, :])
            predT_sb = sm_sb
            bias_sb = sm_sb
            ones_sb = sm_sb

            enc_sb = cp.tile([128, NVT * T], bf16, tag="enc_sb")

            with tc.tile_pool(name="ppsum", bufs=2, space="PSUM") as pp, \
                 tc.tile_pool(name="epsum", bufs=2, space="PSUM") as ep, \
                 tc.tile_pool(name="rep", bufs=3) as rp, \
                 tc.tile_pool(name="stage", bufs=3) as sp:
                for vt in range(NVT):
                    vs = slice(vt * 128, (vt + 1) * 128)
                    # pred_projT[v, u] = pred[u] @ Wp[:, v] + bias[v]
                    psp = pp.tile([128, U1], f32, tag="pp")
                    for c in range(KC):
                        nc.tensor.matmul(
                            psp[:], W_sb[vt][:, 640 + c * 128:640 + (c + 1) * 128],
                            predT_sb[:, c * U1:(c + 1) * U1],
                            start=(c == 0), stop=False)
                    nc.tensor.matmul(
                        psp[:], bias_sb[0:1, PB0 + vt * 128:PB0 + (vt + 1) * 128],
                        ones_sb[0:1, OB0:OB0 + U1],
                        start=False, stop=True)
                    # pred_rep[v, u, r] = pred_projT[v, u] replicated 8x
                    rep = rp.tile([128, U1 * 8], bf16, tag="rep")
                    nc.scalar.copy(
                        rep[:].rearrange("p (u r) -> p u r", r=8),
                        psp[:].unsqueeze(2).broadcast_to([128, U1, 8]))

                    # enc_projT[v, t] = enc[t] @ We[:, v]
                    pse = ep.tile([128, T], f32, tag="pse")
                    for c in range(KC):
                        nc.tensor.matmul(
                            pse[:], W_sb[vt][:, c * 128:(c + 1) * 128],
                            encT_sb[:, c * T:(c + 1) * T],
                            start=(c == 0), stop=(c == KC - 1))
                    esl = enc_sb[:, vt * T:(vt + 1) * T]
                    nc.scalar.copy(esl, pse[:])

                    # broadcast-add + store: one TT + one ~2.1 MB store per half
                    for (u0, nu) in HALVES:
                        st = sp.tile([128, 33 * T], bf16, tag="stage")
                        in0 = (esl.rearrange("p (b r) -> p b r", r=8)
                               .unsqueeze(1).broadcast_to([128, nu, 32, 8]))
                        in1 = (rep[:, u0 * 8:(u0 + nu) * 8]
                               .rearrange("p (u r) -> p u r", r=8)
                               .unsqueeze(2).broadcast_to([128, nu, 32, 8]))
                        outp = st[:, 0:nu * T].rearrange(
                            "p (u b r) -> p u b r", u=nu, b=32, r=8)
                        nc.vector.tensor_add(outp, in0, in1)
                        nc.sync.dma_start(
                            out[vs, u0 * T:(u0 + nu) * T], st[:, 0:nu * T])

    nc.compile()
    return nc


def _get_compiled():
    global _COMPILED
    if _COMPILED is None:
        _COMPILED = _build()
    return _COMPILED


def _bf16(a):
    import ml_dtypes
    return np.ascontiguousarray(a.astype(ml_dtypes.bfloat16))


def _pack_chunks(xT, cols):
    # xT: [D, cols] -> [128, KC*cols] with [p, c*cols+i] = xT[c*128+p, i]
    return xT.reshape(KC, 128, cols).transpose(1, 0, 2).reshape(128, KC * cols)


def _pack_w(Whalf):
    # [c*128+d', vt*128+j] -> [vt*128+d', c*128+j]
    w = np.asarray(Whalf).reshape(KC, 128, NVT, 128)
    return w.transpose(2, 1, 0, 3).reshape(NVT * 128, KC * 128)


def _in_maps(encoder_out, predictor_out, W, b):
    W = np.asarray(W)
    Wall = _bf16(np.concatenate([_pack_w(W[:D]), _pack_w(W[D:])], axis=1))
    maps = []
    for i in range(B):
        sm = np.zeros((128, KC * U1 + V + U1), dtype=np.float32)
        sm[:, :KC * U1] = _pack_chunks(np.asarray(predictor_out[i]).T, U1)
        sm[0, KC * U1:KC * U1 + V] = np.asarray(b)
        sm[0, KC * U1 + V:] = 1.0
        maps.append({
            "encTp": _bf16(_pack_chunks(np.asarray(encoder_out[i]).T, T)),
            "smalls": _bf16(sm),
            "Wall": Wall,
        })
    return maps


def run(encoder_out, predictor_out, W, b, trace=False, tmpdir=None):
    from concourse.bass_utils import run_bass_kernel_spmd

    nc = _get_compiled()
    maps = _in_maps(encoder_out, predictor_out, W, b)
    res = run_bass_kernel_spmd(
        nc, maps, list(range(B)), trace=trace,
        **({"tmpdir": tmpdir} if tmpdir else {}))
    outs = np.empty((B, T, U1, V), dtype=np.float32)
    for i in range(B):
        o = np.asarray(res.results[i]["out"])
        o16 = o.view(np.uint16).reshape(V, U1, T)
        f = (o16.astype(np.uint32) << np.uint32(16)).view(np.float32)
        outs[i] = f.transpose(2, 1, 0)
    return outs, res


def kernel(encoder_out, predictor_out, W, b):
    outs, _ = run(encoder_out, predictor_out, W, b)
    return outs


# revision 17
# speedup vs baseline: 1.1578x; 1.1576x over previous
"""RNN-T Joiner kernel for Trainium2 (Bass/Tile), 8 cores, hybrid
data-parallel (batch pairs) x tensor-parallel (vocab halves).

out[b,t,u,v] = (enc[b,t] @ We)[v] + (pred[b,u] @ Wp)[v] + bias[v]

Core k = (g, h): batches {2g, 2g+1}, vocab half h (512 of 1024). The 2-way
vocab split halves each core's W read (the kernel is DMA-byte-paced, so
total HBM traffic is the metric that matters; output bytes are fixed).

Layout trick: V on partitions, (u, t) on the free dim. The broadcast add is
done by ONE DVE tensor_tensor per u-half with stride-0 broadcast access
patterns ([128, nu, 32, 8]: enc broadcast over u, pred_rep broadcast over
t-blocks, both with packed 2-byte last dims so the DVE 2x mode stays on).
This amortizes the ~120 ns per-instruction overhead over ~8k elements.
GpSimd is deliberately unused: its tensor_scalar is ~4 us/instr on HW and
its SBUF-port contention degrades DVE ~7x (measured).

All inputs are host-packed and all DMAs ride the SP HWDGE ring (loads from
the Act ring measurably unbalance the SDMA engines). Load order: smalls
(pred path + bias), W block 0, encT, W blocks 1-3 — so the first TT issues
~11 us in while the remaining weights stream.

Per core, per (batch, vt) iteration (8 total):
  - PE (bf16): pred_projT [v,65] (+bias rank-1) and enc_projT [v,256] in
    PSUM f32.
  - Act: PSUM->SBUF bf16 copies: enc_projT [128,256] and pred_rep [128,65,8].
  - DVE: one tensor_tensor add per u-half (~4.4 us each at 2x).
  - HWDGE DMA: 2 stores of ~2.1 MB ([b,v,u,t]-order bf16 output).

Output returned to DRAM as bf16 [2*512, U1*T]; host transposes/gathers to
[B,T,U1,V] f32. bf16 end-to-end keeps max rel err ~5e-3, under the 2e-2 gate.
"""

import sys

sys.path.insert(0, "/opt/trn_rl_repo")

import numpy as np

B, T, U1, D, V = 8, 256, 65, 640, 1024
KC = D // 128    # 5 contraction chunks
GB = 2           # batches per core
NVT = 4          # vocab tiles per core (V/2 = 512)
HALVES = [(0, 33), (33, 32)]
PB0 = GB * KC * U1   # bias offset in smalls row 0
OB0 = PB0 + V // 2   # ones offset

_COMPILED = None


def _build():
    import concourse.bacc as bacc
    import concourse.tile as tile
    import concourse.mybir as mybir

    f32 = mybir.dt.float32
    bf16 = mybir.dt.bfloat16

    nc = bacc.Bacc("TRN2", target_bir_lowering=False, debug=False, num_devices=8)

    # host-packed inputs (see _in_maps):
    #   encTp[p, (bl*KC+c)*T+t] = enc[2g+bl][t, c*128+p]
    #   smalls[p, (bl*KC+c)*U1+u] = pred[2g+bl][u, c*128+p];
    #     row 0: bias half at PB0.., ones at OB0..
    #   Wall[vtl*128+p, c*128+j]     = We[c*128+p, (4h+vtl)*128+j] (pred at +640)
    encTp = nc.dram_tensor("encTp", [128, GB * KC * T], bf16,
                           kind="ExternalInput")
    smalls = nc.dram_tensor("smalls", [128, OB0 + U1], bf16,
                            kind="ExternalInput")
    Wall = nc.dram_tensor("Wall", [NVT * 128, 2 * KC * 128], bf16,
                          kind="ExternalInput")
    out = nc.dram_tensor("out", [GB * NVT * 128, U1 * T], bf16,
                         kind="ExternalOutput")

    with tile.TileContext(nc) as tc:
        with tc.tile_pool(name="consts", bufs=1) as cp:
            W_sb = []
            for vtl in range(NVT):
                t_ = cp.tile([128, 2 * KC * 128], bf16, tag=f"W{vtl}")
                W_sb.append(t_)
            sm_sb = cp.tile([128, OB0 + U1], bf16, tag="smalls")
            nc.sync.dma_start(sm_sb[:], smalls[:])
            nc.sync.dma_start(W_sb[0][:], Wall[0:128, :])
            encT_sb = cp.tile([128, GB * KC * T], bf16, tag="encT")
            nc.sync.dma_start(encT_sb[:], encTp[:])
            for vtl in range(1, NVT):
                nc.sync.dma_start(W_sb[vtl][:], Wall[vtl * 128:(vtl + 1) * 128, :])

            enc_sb = cp.tile([128, GB * NVT * T], bf16, tag="enc_sb")

            with tc.tile_pool(name="ppsum", bufs=2, space="PSUM") as pp, \
                 tc.tile_pool(name="epsum", bufs=2, space="PSUM") as ep, \
                 tc.tile_pool(name="rep", bufs=3) as rp, \
                 tc.tile_pool(name="stage", bufs=3) as sp:
                for bl in range(GB):
                    for vtl in range(NVT):
                        idx = bl * NVT + vtl
                        # pred_projT[v, u] = pred[u] @ Wp[:, v] + bias[v]
                        psp = pp.tile([128, U1], f32, tag="pp")
                        for c in range(KC):
                            nc.tensor.matmul(
                                psp[:],
                                W_sb[vtl][:, 640 + c * 128:640 + (c + 1) * 128],
                                sm_sb[:, (bl * KC + c) * U1:(bl * KC + c + 1) * U1],
                                start=(c == 0), stop=False)
                        nc.tensor.matmul(
                            psp[:],
                            sm_sb[0:1, PB0 + vtl * 128:PB0 + (vtl + 1) * 128],
                            sm_sb[0:1, OB0:OB0 + U1],
                            start=False, stop=True)
                        # pred_rep[v, u, r] = pred_projT[v, u] replicated 8x
                        rep = rp.tile([128, U1 * 8], bf16, tag="rep")
                        nc.scalar.copy(
                            rep[:].rearrange("p (u r) -> p u r", r=8),
                            psp[:].unsqueeze(2).broadcast_to([128, U1, 8]))

                        # enc_projT[v, t] = enc[t] @ We[:, v]
                        pse = ep.tile([128, T], f32, tag="pse")
                        for c in range(KC):
                            nc.tensor.matmul(
                                pse[:], W_sb[vtl][:, c * 128:(c + 1) * 128],
                                encT_sb[:, (bl * KC + c) * T:(bl * KC + c + 1) * T],
                                start=(c == 0), stop=(c == KC - 1))
                        esl = enc_sb[:, idx * T:(idx + 1) * T]
                        nc.scalar.copy(esl, pse[:])

                        # broadcast-add + store: one TT + ~2.1 MB store per half
                        for (u0, nu) in HALVES:
                            st = sp.tile([128, 33 * T], bf16, tag="stage")
                            in0 = (esl.rearrange("p (b r) -> p b r", r=8)
                                   .unsqueeze(1).broadcast_to([128, nu, 32, 8]))
                            in1 = (rep[:, u0 * 8:(u0 + nu) * 8]
                                   .rearrange("p (u r) -> p u r", r=8)
                                   .unsqueeze(2).broadcast_to([128, nu, 32, 8]))
                            outp = st[:, 0:nu * T].rearrange(
                                "p (u b r) -> p u b r", u=nu, b=32, r=8)
                            nc.vector.tensor_add(outp, in0, in1)
                            nc.sync.dma_start(
                                out[idx * 128:(idx + 1) * 128, u0 * T:(u0 + nu) * T],
                                st[:, 0:nu * T])

    nc.compile()
    return nc


def _get_compiled():
    global _COMPILED
    if _COMPILED is None:
        _COMPILED = _build()
    return _COMPILED


def _bf16(a):
    import ml_dtypes
    return np.ascontiguousarray(a.astype(ml_dtypes.bfloat16))


def _pack_chunks(xT, cols):
    # xT: [D, cols] -> [128, KC*cols] with [p, c*cols+i] = xT[c*128+p, i]
    return xT.reshape(KC, 128, cols).transpose(1, 0, 2).reshape(128, KC * cols)


def _pack_w(Whalf):
    # [c*128+d', vt*128+j] -> [vt*128+d', c*128+j]  (vt = global, 8 tiles)
    w = np.asarray(Whalf).reshape(KC, 128, 8, 128)
    return w.transpose(2, 1, 0, 3).reshape(8 * 128, KC * 128)


def _in_maps(encoder_out, predictor_out, W, b):
    W = np.asarray(W)
    b = np.asarray(b)
    Wall_full = np.concatenate([_pack_w(W[:D]), _pack_w(W[D:])], axis=1)
    encoder_out = np.asarray(encoder_out)
    predictor_out = np.asarray(predictor_out)
    maps = []
    for k in range(B):
        g, h = divmod(k, 2)
        sm = np.zeros((128, OB0 + U1), dtype=np.float32)
        for bl in range(GB):
            sm[:, bl * KC * U1:(bl + 1) * KC * U1] = _pack_chunks(
                predictor_out[2 * g + bl].T, U1)
        sm[0, PB0:OB0] = b[h * 512:(h + 1) * 512]
        sm[0, OB0:] = 1.0
        enc = np.concatenate(
            [_pack_chunks(encoder_out[2 * g + bl].T, T) for bl in range(GB)],
            axis=1)
        maps.append({
            "encTp": _bf16(enc),
            "smalls": _bf16(sm),
            "Wall": _bf16(Wall_full[h * 512:(h + 1) * 512]),
        })
    return maps


def run(encoder_out, predictor_out, W, b, trace=False, tmpdir=None):
    from concourse.bass_utils import run_bass_kernel_spmd

    nc = _get_compiled()
    maps = _in_maps(encoder_out, predictor_out, W, b)
    res = run_bass_kernel_spmd(
        nc, maps, list(range(B)), trace=trace,
        **({"tmpdir": tmpdir} if tmpdir else {}))
    outs = np.empty((B, T, U1, V), dtype=np.float32)
    for k in range(B):
        g, h = divmod(k, 2)
        o = np.asarray(res.results[k]["out"])
        o16 = o.view(np.uint16).reshape(GB, 512, U1, T)
        f = (o16.astype(np.uint32) << np.uint32(16)).view(np.float32)
        for bl in range(GB):
            outs[2 * g + bl][:, :, h * 512:(h + 1) * 512] = f[bl].transpose(2, 1, 0)
    return outs, res


def kernel(encoder_out, predictor_out, W, b):
    outs, _ = run(encoder_out, predictor_out, W, b)
    return outs


# revision 19
# speedup vs baseline: 1.1680x; 1.0088x over previous
"""RNN-T Joiner kernel for Trainium2 (Bass/Tile), 8 cores, hybrid
data-parallel (batch pairs) x tensor-parallel (vocab halves).

out[b,t,u,v] = (enc[b,t] @ We)[v] + (pred[b,u] @ Wp)[v] + bias[v]

Core k = (g, h): batches {2g, 2g+1}, vocab half h (512 of 1024). The 2-way
vocab split halves each core's W read (the kernel is DMA-byte-paced, so
total HBM traffic is the metric that matters; output bytes are fixed).

Layout trick: V on partitions, (u, t) on the free dim. The broadcast add is
done by ONE DVE tensor_tensor per u-half with stride-0 broadcast access
patterns ([128, nu, 32, 8]: enc broadcast over u, pred_rep broadcast over
t-blocks, both with packed 2-byte last dims so the DVE 2x mode stays on).
This amortizes the ~120 ns per-instruction overhead over ~8k elements.
GpSimd is deliberately unused: its tensor_scalar is ~4 us/instr on HW and
its SBUF-port contention degrades DVE ~7x (measured).

All inputs are host-packed and all DMAs ride the SP HWDGE ring (loads from
the Act ring measurably unbalance the SDMA engines). Load order: smalls
(pred path + bias), W block 0, encT, W blocks 1-3 — so the first TT issues
~11 us in while the remaining weights stream.

Per core, per (batch, vt) iteration (8 total):
  - PE (bf16): pred_projT [v,65] (+bias rank-1) and enc_projT [v,256] in
    PSUM f32.
  - Act: PSUM->SBUF bf16 copies: enc_projT [128,256] and pred_rep [128,65,8].
  - DVE: one tensor_tensor add per u-half (~4.4 us each at 2x).
  - HWDGE DMA: 2 stores of ~2.1 MB ([b,v,u,t]-order bf16 output).

Output returned to DRAM as bf16 [2*512, U1*T]; host transposes/gathers to
[B,T,U1,V] f32. bf16 end-to-end keeps max rel err ~5e-3, under the 2e-2 gate.
"""

import sys

sys.path.insert(0, "/opt/trn_rl_repo")

import numpy as np

B, T, U1, D, V = 8, 256, 65, 640, 1024
KC = D // 128    # 5 contraction chunks
GB = 2           # batches per core
NVT = 4          # vocab tiles per core (V/2 = 512)
HALVES = [(0, 33), (33, 32)]
PB0 = GB * KC * U1   # bias offset in smalls row 0
OB0 = PB0 + V // 2   # ones offset

_COMPILED = None


def _build():
    import concourse.bacc as bacc
    import concourse.tile as tile
    import concourse.mybir as mybir

    f32 = mybir.dt.float32
    bf16 = mybir.dt.bfloat16

    nc = bacc.Bacc("TRN2", target_bir_lowering=False, debug=False, num_devices=8)

    # host-packed inputs (see _in_maps):
    #   encTp[p, (bl*KC+c)*T+t] = enc[2g+bl][t, c*128+p]
    #   smalls[p, (bl*KC+c)*U1+u] = pred[2g+bl][u, c*128+p];
    #     row 0: bias half at PB0.., ones at OB0..
    #   Wall[vtl*128+p, c*128+j]     = We[c*128+p, (4h+vtl)*128+j] (pred at +640)
    encTp = nc.dram_tensor("encTp", [128, GB * KC * T], bf16,
                           kind="ExternalInput")
    smalls = nc.dram_tensor("smalls", [128, OB0 + U1], bf16,
                            kind="ExternalInput")
    Wall = nc.dram_tensor("Wall", [NVT * 128, 2 * KC * 128], bf16,
                          kind="ExternalInput")
    out = nc.dram_tensor("out", [GB * NVT * 128, U1 * T], bf16,
                         kind="ExternalOutput")

    with tile.TileContext(nc) as tc:
        with tc.tile_pool(name="consts", bufs=1) as cp:
            W_sb = []
            for vtl in range(NVT):
                t_ = cp.tile([128, 2 * KC * 128], bf16, tag=f"W{vtl}")
                W_sb.append(t_)
            sm_sb = cp.tile([128, OB0 + U1], bf16, tag="smalls")
            nc.sync.dma_start(sm_sb[:], smalls[:])
            encT_sb = cp.tile([128, GB * KC * T], bf16, tag="encT")
            nc.sync.dma_start(encT_sb[:, 0:KC * T], encTp[:, 0:KC * T])
            nc.sync.dma_start(W_sb[0][:], Wall[0:128, :])
            nc.sync.dma_start(encT_sb[:, KC * T:], encTp[:, KC * T:])
            for vtl in range(1, NVT):
                nc.sync.dma_start(W_sb[vtl][:], Wall[vtl * 128:(vtl + 1) * 128, :])

            enc_sb = cp.tile([128, GB * NVT * T], bf16, tag="enc_sb")

            with tc.tile_pool(name="ppsum", bufs=2, space="PSUM") as pp, \
                 tc.tile_pool(name="epsum", bufs=2, space="PSUM") as ep, \
                 tc.tile_pool(name="rep", bufs=3) as rp, \
                 tc.tile_pool(name="stage", bufs=5) as sp:
                for bl in range(GB):
                    for vtl in range(NVT):
                        idx = bl * NVT + vtl
                        # pred_projT[v, u] = pred[u] @ Wp[:, v] + bias[v]
                        psp = pp.tile([128, U1], f32, tag="pp")
                        for c in range(KC):
                            nc.tensor.matmul(
                                psp[:],
                                W_sb[vtl][:, 640 + c * 128:640 + (c + 1) * 128],
                                sm_sb[:, (bl * KC + c) * U1:(bl * KC + c + 1) * U1],
                                start=(c == 0), stop=False)
                        nc.tensor.matmul(
                            psp[:],
                            sm_sb[0:1, PB0 + vtl * 128:PB0 + (vtl + 1) * 128],
                            sm_sb[0:1, OB0:OB0 + U1],
                            start=False, stop=True)
                        # pred_rep[v, u, r] = pred_projT[v, u] replicated 8x
                        rep = rp.tile([128, U1 * 8], bf16, tag="rep")
                        nc.scalar.copy(
                            rep[:].rearrange("p (u r) -> p u r", r=8),
                            psp[:].unsqueeze(2).broadcast_to([128, U1, 8]))

                        # enc_projT[v, t] = enc[t] @ We[:, v]
                        pse = ep.tile([128, T], f32, tag="pse")
                        for c in range(KC):
                            nc.tensor.matmul(
                                pse[:], W_sb[vtl][:, c * 128:(c + 1) * 128],
                                encT_sb[:, (bl * KC + c) * T:(bl * KC + c + 1) * T],
                                start=(c == 0), stop=(c == KC - 1))
                        esl = enc_sb[:, idx * T:(idx + 1) * T]
                        nc.scalar.copy(esl, pse[:])

                        # broadcast-add + store: one TT + ~2.1 MB store per half
                        for (u0, nu) in HALVES:
                            st = sp.tile([128, 33 * T], bf16, tag="stage")
                            in0 = (esl.rearrange("p (b r) -> p b r", r=8)
                                   .unsqueeze(1).broadcast_to([128, nu, 32, 8]))
                            in1 = (rep[:, u0 * 8:(u0 + nu) * 8]
                                   .rearrange("p (u r) -> p u r", r=8)
                                   .unsqueeze(2).broadcast_to([128, nu, 32, 8]))
                            outp = st[:, 0:nu * T].rearrange(
                                "p (u b r) -> p u b r", u=nu, b=32, r=8)
                            nc.vector.tensor_add(outp, in0, in1)
                            nc.sync.dma_start(
                                out[idx * 128:(idx + 1) * 128, u0 * T:(u0 + nu) * T],
                                st[:, 0:nu * T])

    nc.compile()
    return nc


def _get_compiled():
    global _COMPILED
    if _COMPILED is None:
        _COMPILED = _build()
    return _COMPILED


def _bf16(a):
    import ml_dtypes
    return np.ascontiguousarray(a.astype(ml_dtypes.bfloat16))


def _pack_chunks(xT, cols):
    # xT: [D, cols] -> [128, KC*cols] with [p, c*cols+i] = xT[c*128+p, i]
    return xT.reshape(KC, 128, cols).transpose(1, 0, 2).reshape(128, KC * cols)


def _pack_w(Whalf):
    # [c*128+d', vt*128+j] -> [vt*128+d', c*128+j]  (vt = global, 8 tiles)
    w = np.asarray(Whalf).reshape(KC, 128, 8, 128)
    return w.transpose(2, 1, 0, 3).reshape(8 * 128, KC * 128)


def _in_maps(encoder_out, predictor_out, W, b):
    W = np.asarray(W)
    b = np.asarray(b)
    Wall_full = np.concatenate([_pack_w(W[:D]), _pack_w(W[D:])], axis=1)
    encoder_out = np.asarray(encoder_out)
    predictor_out = np.asarray(predictor_out)
    maps = []
    for k in range(B):
        g, h = divmod(k, 2)
        sm = np.zeros((128, OB0 + U1), dtype=np.float32)
        for bl in range(GB):
            sm[:, bl * KC * U1:(bl + 1) * KC * U1] = _pack_chunks(
                predictor_out[2 * g + bl].T, U1)
        sm[0, PB0:OB0] = b[h * 512:(h + 1) * 512]
        sm[0, OB0:] = 1.0
        enc = np.concatenate(
            [_pack_chunks(encoder_out[2 * g + bl].T, T) for bl in range(GB)],
            axis=1)
        maps.append({
            "encTp": _bf16(enc),
            "smalls": _bf16(sm),
            "Wall": _bf16(Wall_full[h * 512:(h + 1) * 512]),
        })
    return maps


def run(encoder_out, predictor_out, W, b, trace=False, tmpdir=None):
    from concourse.bass_utils import run_bass_kernel_spmd

    nc = _get_compiled()
    maps = _in_maps(encoder_out, predictor_out, W, b)
    res = run_bass_kernel_spmd(
        nc, maps, list(range(B)), trace=trace,
        **({"tmpdir": tmpdir} if tmpdir else {}))
    outs = np.empty((B, T, U1, V), dtype=np.float32)
    for k in range(B):
        g, h = divmod(k, 2)
        o = np.asarray(res.results[k]["out"])
        o16 = o.view(np.uint16).reshape(GB, 512, U1, T)
        f = (o16.astype(np.uint32) << np.uint32(16)).view(np.float32)
        for bl in range(GB):
            outs[2 * g + bl][:, :, h * 512:(h + 1) * 512] = f[bl].transpose(2, 1, 0)
    return outs, res


def kernel(encoder_out, predictor_out, W, b):
    outs, _ = run(encoder_out, predictor_out, W, b)
    return outs


# revision 21
# speedup vs baseline: 1.1793x; 1.0097x over previous
"""RNN-T Joiner kernel for Trainium2 (Bass/Tile), 8 cores, hybrid
data-parallel (batch pairs) x tensor-parallel (vocab halves).

out[b,t,u,v] = (enc[b,t] @ We)[v] + (pred[b,u] @ Wp)[v] + bias[v]

Core k = (g, h): batches {2g, 2g+1}, vocab half h (512 of 1024). The 2-way
vocab split halves each core's W read (the kernel is DMA-byte-paced, so
total HBM traffic is the metric that matters; output bytes are fixed).

Layout trick: V on partitions, (u, t) on the free dim. The broadcast add is
done by ONE DVE tensor_tensor per u-half with stride-0 broadcast access
patterns ([128, nu, 32, 8]: enc broadcast over u, pred_rep broadcast over
t-blocks, both with packed 2-byte last dims so the DVE 2x mode stays on).
This amortizes the ~120 ns per-instruction overhead over ~8k elements.
GpSimd is deliberately unused: its tensor_scalar is ~4 us/instr on HW and
its SBUF-port contention degrades DVE ~7x (measured).

All inputs are host-packed and all DMAs ride the SP HWDGE ring (loads from
the Act ring measurably unbalance the SDMA engines). Load order: smalls
(pred path + bias), W block 0, encT, W blocks 1-3 — so the first TT issues
~11 us in while the remaining weights stream.

Per core, per (batch, vt) iteration (8 total):
  - PE (bf16): pred_projT [v,65] (+bias rank-1) and enc_projT [v,256] in
    PSUM f32.
  - Act: PSUM->SBUF bf16 copies: enc_projT [128,256] and pred_rep [128,65,8].
  - DVE: one tensor_tensor add per u-half (~4.4 us each at 2x).
  - HWDGE DMA: 2 stores of ~2.1 MB ([b,v,u,t]-order bf16 output).

Output returned to DRAM as bf16 [2*512, U1*T]; host transposes/gathers to
[B,T,U1,V] f32. bf16 end-to-end keeps max rel err ~5e-3, under the 2e-2 gate.
"""

import sys

sys.path.insert(0, "/opt/trn_rl_repo")

import numpy as np

B, T, U1, D, V = 8, 256, 65, 640, 1024
KC = D // 128    # 5 contraction chunks
GB = 2           # batches per core
NVT = 4          # vocab tiles per core (V/2 = 512)
HALVES = [(0, 33), (33, 32)]
PB0 = GB * KC * U1   # bias offset in smalls row 0
OB0 = PB0 + V // 2   # ones offset

_COMPILED = None


def _build():
    import concourse.bacc as bacc
    import concourse.tile as tile
    import concourse.mybir as mybir

    f32 = mybir.dt.float32
    bf16 = mybir.dt.bfloat16

    nc = bacc.Bacc("TRN2", target_bir_lowering=False, debug=False, num_devices=8)

    # host-packed inputs (see _in_maps):
    #   encTp[p, (bl*KC+c)*T+t] = enc[2g+bl][t, c*128+p]
    #   smalls[p, (bl*KC+c)*U1+u] = pred[2g+bl][u, c*128+p];
    #     row 0: bias half at PB0.., ones at OB0..
    #   Wall[vtl*128+p, c*128+j]     = We[c*128+p, (4h+vtl)*128+j] (pred at +640)
    encTp = nc.dram_tensor("encTp", [128, GB * KC * T], bf16,
                           kind="ExternalInput")
    smalls = nc.dram_tensor("smalls", [128, OB0 + U1], bf16,
                            kind="ExternalInput")
    Wall = nc.dram_tensor("Wall", [NVT * 128, 2 * KC * 128], bf16,
                          kind="ExternalInput")
    out = nc.dram_tensor("out", [GB * NVT * 128, U1 * T], bf16,
                         kind="ExternalOutput")

    with tile.TileContext(nc) as tc:
        with tc.tile_pool(name="consts", bufs=1) as cp:
            W_sb = []
            for vtl in range(NVT):
                t_ = cp.tile([128, 2 * KC * 128], bf16, tag=f"W{vtl}")
                W_sb.append(t_)
            # W0 is split pred-half-first: the pred matmul chain only reads
            # cols 640+, so sub-tile deps let PE start ~1.5 us earlier
            sm_sb = cp.tile([128, OB0 + U1], bf16, tag="smalls")
            nc.sync.dma_start(W_sb[0][:, 640:], Wall[0:128, 640:])
            nc.sync.dma_start(sm_sb[:], smalls[:])
            nc.sync.dma_start(W_sb[0][:, 0:640], Wall[0:128, 0:640])
            encT_sb = cp.tile([128, GB * KC * T], bf16, tag="encT")
            nc.sync.dma_start(encT_sb[:, 0:KC * T], encTp[:, 0:KC * T])
            nc.sync.dma_start(encT_sb[:, KC * T:], encTp[:, KC * T:])
            for vtl in range(1, NVT):
                nc.sync.dma_start(W_sb[vtl][:], Wall[vtl * 128:(vtl + 1) * 128, :])

            enc_sb = cp.tile([128, GB * NVT * T], bf16, tag="enc_sb")

            with tc.tile_pool(name="ppsum", bufs=2, space="PSUM") as pp, \
                 tc.tile_pool(name="epsum", bufs=2, space="PSUM") as ep, \
                 tc.tile_pool(name="rep", bufs=3) as rp, \
                 tc.tile_pool(name="stage", bufs=5) as sp:
                for bl in range(GB):
                    for vtl in range(NVT):
                        idx = bl * NVT + vtl
                        # pred_projT[v, u] = pred[u] @ Wp[:, v] + bias[v]
                        psp = pp.tile([128, U1], f32, tag="pp")
                        for c in range(KC):
                            nc.tensor.matmul(
                                psp[:],
                                W_sb[vtl][:, 640 + c * 128:640 + (c + 1) * 128],
                                sm_sb[:, (bl * KC + c) * U1:(bl * KC + c + 1) * U1],
                                start=(c == 0), stop=False)
                        nc.tensor.matmul(
                            psp[:],
                            sm_sb[0:1, PB0 + vtl * 128:PB0 + (vtl + 1) * 128],
                            sm_sb[0:1, OB0:OB0 + U1],
                            start=False, stop=True)
                        # pred_rep[v, u, r] = pred_projT[v, u] replicated 8x
                        rep = rp.tile([128, U1 * 8], bf16, tag="rep")
                        nc.scalar.copy(
                            rep[:].rearrange("p (u r) -> p u r", r=8),
                            psp[:].unsqueeze(2).broadcast_to([128, U1, 8]))

                        # enc_projT[v, t] = enc[t] @ We[:, v]
                        pse = ep.tile([128, T], f32, tag="pse")
                        for c in range(KC):
                            nc.tensor.matmul(
                                pse[:], W_sb[vtl][:, c * 128:(c + 1) * 128],
                                encT_sb[:, (bl * KC + c) * T:(bl * KC + c + 1) * T],
                                start=(c == 0), stop=(c == KC - 1))
                        esl = enc_sb[:, idx * T:(idx + 1) * T]
                        nc.scalar.copy(esl, pse[:])

                        # broadcast-add + store: one TT + ~2.1 MB store per half
                        # (first iteration uses ~1 MB quarters so the store
                        # stream starts ~2.5 us earlier)
                        blocks = ([(0, 16), (16, 17), (33, 16), (49, 16)]
                                  if idx == 0 else HALVES)
                        for (u0, nu) in blocks:
                            st = sp.tile([128, 33 * T], bf16, tag="stage")
                            in0 = (esl.rearrange("p (b r) -> p b r", r=8)
                                   .unsqueeze(1).broadcast_to([128, nu, 32, 8]))
                            in1 = (rep[:, u0 * 8:(u0 + nu) * 8]
                                   .rearrange("p (u r) -> p u r", r=8)
                                   .unsqueeze(2).broadcast_to([128, nu, 32, 8]))
                            outp = st[:, 0:nu * T].rearrange(
                                "p (u b r) -> p u b r", u=nu, b=32, r=8)
                            nc.vector.tensor_add(outp, in0, in1)
                            nc.sync.dma_start(
                                out[idx * 128:(idx + 1) * 128, u0 * T:(u0 + nu) * T],
                                st[:, 0:nu * T])

    nc.compile()
    return nc


def _get_compiled():
    global _COMPILED
    if _COMPILED is None:
        _COMPILED = _build()
    return _COMPILED


def _bf16(a):
    import ml_dtypes
    return np.ascontiguousarray(a.astype(ml_dtypes.bfloat16))


def _pack_chunks(xT, cols):
    # xT: [D, cols] -> [128, KC*cols] with [p, c*cols+i] = xT[c*128+p, i]
    return xT.reshape(KC, 128, cols).transpose(1, 0, 2).reshape(128, KC * cols)


def _pack_w(Whalf):
    # [c*128+d', vt*128+j] -> [vt*128+d', c*128+j]  (vt = global, 8 tiles)
    w = np.asarray(Whalf).reshape(KC, 128, 8, 128)
    return w.transpose(2, 1, 0, 3).reshape(8 * 128, KC * 128)


def _in_maps(encoder_out, predictor_out, W, b):
    W = np.asarray(W)
    b = np.asarray(b)
    Wall_full = np.concatenate([_pack_w(W[:D]), _pack_w(W[D:])], axis=1)
    encoder_out = np.asarray(encoder_out)
    predictor_out = np.asarray(predictor_out)
    maps = []
    for k in range(B):
        g, h = divmod(k, 2)
        sm = np.zeros((128, OB0 + U1), dtype=np.float32)
        for bl in range(GB):
            sm[:, bl * KC * U1:(bl + 1) * KC * U1] = _pack_chunks(
                predictor_out[2 * g + bl].T, U1)
        sm[0, PB0:OB0] = b[h * 512:(h + 1) * 512]
        sm[0, OB0:] = 1.0
        enc = np.concatenate(
            [_pack_chunks(encoder_out[2 * g + bl].T, T) for bl in range(GB)],
            axis=1)
        maps.append({
            "encTp": _bf16(enc),
            "smalls": _bf16(sm),
            "Wall": _bf16(Wall_full[h * 512:(h + 1) * 512]),
        })
    return maps


def run(encoder_out, predictor_out, W, b, trace=False, tmpdir=None):
    from concourse.bass_utils import run_bass_kernel_spmd

    nc = _get_compiled()
    maps = _in_maps(encoder_out, predictor_out, W, b)
    res = run_bass_kernel_spmd(
        nc, maps, list(range(B)), trace=trace,
        **({"tmpdir": tmpdir} if tmpdir else {}))
    outs = np.empty((B, T, U1, V), dtype=np.float32)
    for k in range(B):
        g, h = divmod(k, 2)
        o = np.asarray(res.results[k]["out"])
        o16 = o.view(np.uint16).reshape(GB, 512, U1, T)
        f = (o16.astype(np.uint32) << np.uint32(16)).view(np.float32)
        for bl in range(GB):
            outs[2 * g + bl][:, :, h * 512:(h + 1) * 512] = f[bl].transpose(2, 1, 0)
    return outs, res


def kernel(encoder_out, predictor_out, W, b):
    outs, _ = run(encoder_out, predictor_out, W, b)
    return outs
